# revision 40
# baseline (speedup 1.0000x reference)
"""Trainium2 Bass kernel for nn_Attention_75342316306884.

Per-batch channel-channel attention:
  xf = x.reshape(B, C, HW); cf = condition.reshape(B, C, HW)
  w1 = softmax(xf @ xf^T * HW^-0.5); w2 = softmax(sig(cf) @ sig(cf)^T * HW^-0.5)
  out = xf + (w1 + w2) @ xf          -> [B, C, HW] float32

Sharding: pure data parallel, batch dim 64 -> 8 cores x 8 batches.

Per-core pipeline, software-skewed two batches ahead (emission order
stage(b+2), gram(b+1), apply(b) so staging and grams overlap applies in
every engine stream):
  stage: one cast-DMA per tensor (f32 HBM -> bf16 [128, 4, 896] SBUF tile
    via a 3D access pattern; pad cols unused); condition -> sigmoid via
    tanh (sigmoid(z) = 0.5*tanh(z/2) + 0.5 -- tanh lives in the same ACT
    table set as exp: zero ACT table switches in the whole kernel); then
    ONE SBUF->SBUF xbar DMA-transpose per tensor into a [128, 28, 512->128]
    bf16 tile: slice [:, 7*cb + e, :] holds rows n = 128e+p of block cb.
  gram: two 512x512 grams on TensorE (bf16, f32 PSUM accumulate), the
    contraction walking 7 chunk-slices (the moving operand reads the
    transposed tile with a stride-7 middle dim); ACT exp with fused
    per-row accumulation Z; diag(Z1) tiles built on ACT.
  apply: 9+4 matmuls per (c-block) into 2-bank [128, 1024] PSUM tiles with
    the residual folded in as an extra diag(Z1) matmul chunk; epilogue is
    t1 = u1*r1 (DVE), t2 = u2*r2 (ACT), o = t1+t2 (DVE, bf16) and one
    cast-DMA per batch back to f32 HBM.

Key algebraic tricks:
 * G = xf@xf^T is symmetric, so unnormalized E = exp(G*s) is symmetric too;
   softmax(G) @ xf == diag(1/rowsum(E)) @ (E @ xf).  E's stored
   [c-part, d-free] tiles serve directly as the [K=d, M=c] stationary
   operands of the apply matmul -- no attention-matrix transpose, and the
   normalization is a per-partition scalar.  (exp without max-subtraction
   is safe: logits bounded by ~|x|^2/28 ~ 35.)
 * residual: out = (E1 @ X + diag(Z1) @ X) * r1 + (E2 @ X) * r2, so the
   "+ xf" is one extra 128-contraction matmul instead of a VectorE pass.
"""

import sys

import numpy as np

for _p in ("/opt/trn_rl_repo",):
    if _p not in sys.path:
        sys.path.append(_p)

import ml_dtypes

import concourse.bass as bass
import concourse.mybir as mybir
import concourse.tile as tile
from concourse.bass_utils import run_bass_kernel_spmd
from concourse.vector_clock import ScopedClock

F32 = mybir.dt.float32
BF16 = mybir.dt.bfloat16
AF = mybir.ActivationFunctionType
MUL = mybir.AluOpType.mult
ADD = mybir.AluOpType.add

N_CORES = 8
B_PER_CORE = 8
C = 512  # channels
HW = 784  # 28*28
HWP = 896  # padded to 7*128 for the xbar transpose
SCALE = float(HW) ** -0.5
P = 128
N_KCH = 7  # gram contraction chunks: 6x128 + 1x16
KCH_SIZES = (128, 128, 128, 128, 128, 128, 16)
N_CB = 4  # 512 / 128 c-blocks
APPLY_NSPLIT = ((0, 512), (512, 272))


def _patch_tile_drain():
    """walrus codegen in this toolchain rejects >1 sem-wait on one SP CTRL
    (drain/nop) instruction; spread the Tile end-of-context drain waits
    across several nops instead."""
    if getattr(tile.TileContext, "_drain_patched", False):
        return

    def _drain_and_barrier(self, tick_clock, wait_clock):
        absorber = self.nc.sync.nop()
        wait_clock.add_sem_waits(
            absorber.ins, ScopedClock({None: tick_clock.global_clock})
        )
        si = absorber.ins.sync_info
        waits = list(si.on_wait) if si is not None and si.on_wait else []
        if len(waits) > 1:
            absorber.ins.sync_info = mybir.SyncInfo(on_wait=waits[:1], on_update=[])
            for w in waits[1:]:
                n2 = self.nc.sync.nop()
                n2.ins.sync_info = mybir.SyncInfo(on_wait=[w], on_update=[])
        self.nc.sync.drain()
        self.nc.all_engine_barrier()
        assert self.sems is not None
        popped = self.nc._tile_sem_poison_stack.pop()
        assert popped is self._sem_poison
        self.nc.clear_and_free_semaphores(list(self.sems.allocated().values()))
        self.nc.all_engine_barrier()

    tile.TileContext._drain_and_barrier = _drain_and_barrier
    tile.TileContext._drain_patched = True


def _split_multi_waits(nc, limit=1):
    """This walrus build allows only `limit` sem-wait commands per
    instruction.  Hoist excess waits onto same-engine NoOps placed
    immediately before the instruction (per-engine program order makes
    this semantically identical)."""
    n_split = 0
    for f in nc.m.functions:
        for bb in f.blocks:
            new_insts = []
            for inst in bb.instructions:
                si = inst.sync_info
                waits = list(si.on_wait) if si is not None and si.on_wait else []
                if len(waits) > limit:
                    for j, w in enumerate(waits[:-limit]):
                        nop = mybir.InstNoOp(
                            name=f"{inst.name}-wsplit{j}", ins=[], outs=[]
                        )
                        nop.engine = inst.engine
                        nop.sync_info = mybir.SyncInfo(on_wait=[w], on_update=[])
                        new_insts.append(nop)
                    inst.sync_info = mybir.SyncInfo(
                        on_wait=waits[-limit:],
                        on_update=list(si.on_update) if si.on_update else [],
                    )
                    n_split += 1
                new_insts.append(inst)
            if len(new_insts) != len(bb.instructions):
                bb.instructions = new_insts
                assert len(bb.instructions) == len(new_insts)
    return n_split


def _gram_exp(nc, psum_g, opT, e_pool, z_pool, etag):
    """opT: one [128, 7, 512] bf16 tile; chunk e's partition p holds row
    n = 128e + p of the transposed operand (chunk 6: first 16 valid).
    Returns (E, z, r): E = exp(scale*gram) (4 x [128, 512] bf16),
    z = rowsum(E), r = 1/z (each 4 x [128, 1] f32)."""
    es, zs, rs = [], [], []
    for cb in range(N_CB):
        g = psum_g.tile([P, C], F32, tag="g")
        for i, k in enumerate(range(N_KCH)):
            kk = KCH_SIZES[k]
            nc.tensor.matmul(
                g[:],
                opT[:kk, N_KCH * cb + k, :],
                opT[:kk, k :: N_KCH, :],
                start=(i == 0),
                stop=(i == N_KCH - 1),
            )
        e = e_pool.tile([P, C], BF16, tag=etag)
        z = z_pool.tile([P, 1], F32, tag="z" + etag)
        nc.scalar.activation(e[:], g[:], AF.Exp, scale=SCALE, accum_out=z[:])
        r = z_pool.tile([P, 1], F32, tag="r" + etag)
        nc.vector.reciprocal(r[:], z[:])
        es.append(e)
        zs.append(z)
        rs.append(r)
    return es, zs, rs


def build_kernel():
    _patch_tile_drain()
    nc = bass.Bass()
    x_ext = nc.declare_dram_parameter("x", [B_PER_CORE, C, HW], F32, isOutput=False)
    c_ext = nc.declare_dram_parameter(
        "condition", [B_PER_CORE, C, HW], F32, isOutput=False
    )
    out_ext = nc.declare_dram_parameter("out", [B_PER_CORE, C, HW], F32, isOutput=True)

    eye_dram = nc.inline_tensor(np.eye(P, dtype=ml_dtypes.bfloat16), name="eye128")

    with tile.TileContext(nc) as tc:
        with (
            tc.tile_pool(name="const", bufs=1) as const_pool,
            tc.tile_pool(name="xn", bufs=3) as xn_pool,
            tc.tile_pool(name="cn", bufs=3) as cn_pool,
            tc.tile_pool(name="cs", bufs=3) as cs_pool,
            tc.tile_pool(name="xT", bufs=3) as xT_pool,
            tc.tile_pool(name="csT", bufs=3) as csT_pool,
            tc.tile_pool(name="E", bufs=20) as e_pool,
            tc.tile_pool(name="z", bufs=24) as z_pool,
            tc.tile_pool(name="D", bufs=6) as d_pool,
            tc.tile_pool(name="outs", bufs=4) as out_pool,
            tc.tile_pool(name="psum_g", bufs=4, space="PSUM") as psum_g,
            tc.tile_pool(name="psum_u", bufs=2, space="PSUM") as psum_u,
        ):
            eye = const_pool.tile([P, P], BF16)
            nc.sync.dma_start(eye[:], eye_dram[:])

            staged = {}
            grams = {}

            def stage(b):
                """loads + sigmoid-via-tanh + SBUF->SBUF xbar transposes."""
                xTb = xT_pool.tile([P, N_CB * N_KCH, P], BF16, tag="xT")
                csTb = csT_pool.tile([P, N_CB * N_KCH, P], BF16, tag="csT")
                # one cast-DMA for all of x[b]: [128, 4, 896] bf16 (pad unused)
                xnb = xn_pool.tile([P, N_CB, HWP], BF16, tag="xn")
                nc.gpsimd.dma_start(
                    xnb[:, :, :HW],
                    x_ext[b].rearrange("(k p) n -> p k n", p=P),
                )
                nc.sync.dma_start_transpose(
                    xTb[:], xnb.rearrange("p k n -> p (k n)")
                )
                cnb = cn_pool.tile([P, N_CB, HW], BF16, tag="cn")
                nc.gpsimd.dma_start(
                    cnb[:], c_ext[b].rearrange("(k p) n -> p k n", p=P)
                )
                csb = cs_pool.tile([P, N_CB, HWP], BF16, tag="cs")
                # sigmoid(z) = 0.5 * tanh(z/2) + 0.5 (same ACT set as exp)
                ct = cs_pool.tile([P, N_CB, HW], BF16, tag="ct")
                nc.scalar.activation(ct[:], cnb[:], AF.Tanh, scale=0.5)
                nc.vector.tensor_scalar(csb[:, :, :HW], ct[:], 0.5, 0.5, MUL, ADD)
                nc.sync.dma_start_transpose(
                    csTb[:], csb.rearrange("p k n -> p (k n)")
                )
                staged[b] = (xnb, xTb, csTb)

            def gram_stage(b):
                xnb, xTb, csTb = staged.pop(b)
                E1, z1, r1 = _gram_exp(nc, psum_g, xTb, e_pool, z_pool, "e1")
                E2, _z2, r2 = _gram_exp(nc, psum_g, csTb, e_pool, z_pool, "e2")

                # D[cb] = diag(Z1[cb]) as a [128, 128] bf16 tile
                Ds = []
                for cb in range(N_CB):
                    d = d_pool.tile([P, P], BF16, tag="D")
                    nc.scalar.activation(d[:], eye[:], AF.Copy, scale=z1[cb][:])
                    Ds.append(d)
                grams[b] = (xnb, E1, r1, E2, r2, Ds)

            def apply_stage(b):
                xnb, E1, r1, E2, r2, Ds = grams.pop(b)
                obig = out_pool.tile([P, N_CB, HW], BF16, tag="o")
                for cb in range(N_CB):
                    o = obig[:, cb, :]
                    u1 = psum_u.tile([P, 1024], F32, tag="u")
                    u2 = psum_u.tile([P, 1024], F32, tag="u")
                    for n0, nw in APPLY_NSPLIT:
                        for k in range(N_CB):
                            nc.tensor.matmul(
                                u1[:, n0 : n0 + nw],
                                E1[k][:, cb * P : (cb + 1) * P],
                                xnb[:, k, n0 : n0 + nw],
                                start=(k == 0),
                                stop=False,
                            )
                        # residual: diag(Z1) @ X so that u1*r1 includes +X
                        nc.tensor.matmul(
                            u1[:, n0 : n0 + nw],
                            Ds[cb][:],
                            xnb[:, cb, n0 : n0 + nw],
                            start=False,
                            stop=True,
                        )
                        for k in range(N_CB):
                            nc.tensor.matmul(
                                u2[:, n0 : n0 + nw],
                                E2[k][:, cb * P : (cb + 1) * P],
                                xnb[:, k, n0 : n0 + nw],
                                start=(k == 0),
                                stop=(k == N_CB - 1),
                            )
                    t1 = out_pool.tile([P, HW], BF16, tag="t1")
                    nc.vector.tensor_scalar(t1[:], u1[:, :HW], r1[cb][:], None, MUL)
                    t2 = out_pool.tile([P, HW], BF16, tag="t2")
                    nc.scalar.activation(t2[:], u2[:, :HW], AF.Copy, scale=r2[cb][:])
                    nc.vector.tensor_add(o[:], t1[:], t2[:])
                nc.gpsimd.dma_start(
                    out_ext[b].rearrange("(k p) n -> p k n", p=P), obig[:]
                )

            stage(0)
            stage(1)
            gram_stage(0)
            for b in range(B_PER_CORE):
                if b + 2 < B_PER_CORE:
                    stage(b + 2)
                if b + 1 < B_PER_CORE:
                    gram_stage(b + 1)
                apply_stage(b)
    n = _split_multi_waits(nc)
    print(f"[kernel] split {n} multi-wait instructions")
    return nc


_NC_CACHE = None


def kernel(x: np.ndarray, condition: np.ndarray, _trace: bool = False):
    """Full inputs [64, 512, 28, 28] f32 -> full output [64, 512, 784] f32."""
    global _NC_CACHE
    B = x.shape[0]
    xf = np.ascontiguousarray(x.reshape(B, C, HW), dtype=np.float32)
    cf = np.ascontiguousarray(condition.reshape(B, C, HW), dtype=np.float32)

    if _NC_CACHE is None:
        _NC_CACHE = build_kernel()
    nc = _NC_CACHE

    in_maps = [
        {
            "x": xf[i * B_PER_CORE : (i + 1) * B_PER_CORE],
            "condition": cf[i * B_PER_CORE : (i + 1) * B_PER_CORE],
        }
        for i in range(N_CORES)
    ]
    res = run_bass_kernel_spmd(nc, in_maps, core_ids=list(range(N_CORES)), trace=_trace)
    out = np.concatenate([res.results[i]["out"] for i in range(N_CORES)], axis=0)
    kernel.last_result = res
    return out


# revision 42
# speedup vs baseline: 1.1406x; 1.1406x over previous
"""Trainium2 Bass kernel for nn_Attention_75342316306884.

Per-batch channel-channel attention:
  xf = x.reshape(B, C, HW); cf = condition.reshape(B, C, HW)
  w1 = softmax(xf @ xf^T * HW^-0.5); w2 = softmax(sig(cf) @ sig(cf)^T * HW^-0.5)
  out = xf + (w1 + w2) @ xf          -> [B, C, HW] float32

Sharding: pure data parallel, batch dim 64 -> 8 cores x 8 batches.

Per-core pipeline, software-skewed two batches ahead (emission order
stage(b+2), gram(b+1), apply(b) so staging and grams overlap applies in
every engine stream):
  stage: one cast-DMA per tensor (f32 HBM -> bf16 [128, 4, 896] SBUF tile
    via a 3D access pattern; pad cols unused); condition -> sigmoid via
    tanh (sigmoid(z) = 0.5*tanh(z/2) + 0.5 -- tanh lives in the same ACT
    table set as exp: zero ACT table switches in the whole kernel); then
    ONE SBUF->SBUF xbar DMA-transpose per tensor into a [128, 28, 512->128]
    bf16 tile: slice [:, 7*cb + e, :] holds rows n = 128e+p of block cb.
  gram: two 512x512 grams on TensorE (bf16, f32 PSUM accumulate), the
    contraction walking 7 chunk-slices (the moving operand reads the
    transposed tile with a stride-7 middle dim); ACT exp with fused
    per-row accumulation Z; diag(Z1) tiles built on ACT.
  apply: 9+4 matmuls per (c-block) into 2-bank [128, 1024] PSUM tiles with
    the residual folded in as an extra diag(Z1) matmul chunk; epilogue is
    t1 = u1*r1 (DVE), t2 = u2*r2 (ACT), o = t1+t2 (DVE, bf16) and one
    cast-DMA per batch back to f32 HBM.

Key algebraic tricks:
 * G = xf@xf^T is symmetric, so unnormalized E = exp(G*s) is symmetric too;
   softmax(G) @ xf == diag(1/rowsum(E)) @ (E @ xf).  E's stored
   [c-part, d-free] tiles serve directly as the [K=d, M=c] stationary
   operands of the apply matmul -- no attention-matrix transpose, and the
   normalization is a per-partition scalar.  (exp without max-subtraction
   is safe: logits bounded by ~|x|^2/28 ~ 35.)
 * residual: out = (E1 @ X + diag(Z1) @ X) * r1 + (E2 @ X) * r2, so the
   "+ xf" is one extra 128-contraction matmul instead of a VectorE pass.
"""

import sys

import numpy as np

for _p in ("/opt/trn_rl_repo",):
    if _p not in sys.path:
        sys.path.append(_p)

import ml_dtypes

import concourse.bass as bass
import concourse.mybir as mybir
import concourse.tile as tile
from concourse.bass_utils import run_bass_kernel_spmd
from concourse.vector_clock import ScopedClock

F32 = mybir.dt.float32
BF16 = mybir.dt.bfloat16
AF = mybir.ActivationFunctionType
MUL = mybir.AluOpType.mult
ADD = mybir.AluOpType.add

N_CORES = 8
B_PER_CORE = 8
C = 512  # channels
HW = 784  # 28*28
HWP = 896  # padded to 7*128 for the xbar transpose
SCALE = float(HW) ** -0.5
P = 128
N_KCH = 7  # gram contraction chunks: 6x128 + 1x16
KCH_SIZES = (128, 128, 128, 128, 128, 128, 16)
N_CB = 4  # 512 / 128 c-blocks
APPLY_NSPLIT = ((0, 512), (512, 272))


def _patch_tile_drain():
    """walrus codegen in this toolchain rejects >1 sem-wait on one SP CTRL
    (drain/nop) instruction; spread the Tile end-of-context drain waits
    across several nops instead."""
    if getattr(tile.TileContext, "_drain_patched", False):
        return

    def _drain_and_barrier(self, tick_clock, wait_clock):
        absorber = self.nc.sync.nop()
        wait_clock.add_sem_waits(
            absorber.ins, ScopedClock({None: tick_clock.global_clock})
        )
        si = absorber.ins.sync_info
        waits = list(si.on_wait) if si is not None and si.on_wait else []
        if len(waits) > 1:
            absorber.ins.sync_info = mybir.SyncInfo(on_wait=waits[:1], on_update=[])
            for w in waits[1:]:
                n2 = self.nc.sync.nop()
                n2.ins.sync_info = mybir.SyncInfo(on_wait=[w], on_update=[])
        self.nc.sync.drain()
        self.nc.all_engine_barrier()
        assert self.sems is not None
        popped = self.nc._tile_sem_poison_stack.pop()
        assert popped is self._sem_poison
        self.nc.clear_and_free_semaphores(list(self.sems.allocated().values()))
        self.nc.all_engine_barrier()

    tile.TileContext._drain_and_barrier = _drain_and_barrier
    tile.TileContext._drain_patched = True


def _split_multi_waits(nc, limit=1):
    """This walrus build allows only `limit` sem-wait commands per
    instruction.  Hoist excess waits onto same-engine NoOps placed
    immediately before the instruction (per-engine program order makes
    this semantically identical)."""
    n_split = 0
    for f in nc.m.functions:
        for bb in f.blocks:
            new_insts = []
            for inst in bb.instructions:
                si = inst.sync_info
                waits = list(si.on_wait) if si is not None and si.on_wait else []
                if len(waits) > limit:
                    for j, w in enumerate(waits[:-limit]):
                        nop = mybir.InstNoOp(
                            name=f"{inst.name}-wsplit{j}", ins=[], outs=[]
                        )
                        nop.engine = inst.engine
                        nop.sync_info = mybir.SyncInfo(on_wait=[w], on_update=[])
                        new_insts.append(nop)
                    inst.sync_info = mybir.SyncInfo(
                        on_wait=waits[-limit:],
                        on_update=list(si.on_update) if si.on_update else [],
                    )
                    n_split += 1
                new_insts.append(inst)
            if len(new_insts) != len(bb.instructions):
                bb.instructions = new_insts
                assert len(bb.instructions) == len(new_insts)
    return n_split


def _gram_exp(nc, psum_g, opT, e_pool, z_pool, etag):
    """opT: one [128, 7, 512] bf16 tile; chunk e's partition p holds row
    n = 128e + p of the transposed operand (chunk 6: first 16 valid).
    Returns (E, z, r): E = exp(scale*gram) (4 x [128, 512] bf16),
    z = rowsum(E), r = 1/z (each 4 x [128, 1] f32)."""
    es, zs, rs = [], [], []
    for cb in range(N_CB):
        g = psum_g.tile([P, C], F32, tag="g")
        for i, k in enumerate(range(N_KCH)):
            kk = KCH_SIZES[k]
            nc.tensor.matmul(
                g[:],
                opT[:kk, N_KCH * cb + k, :],
                opT[:kk, k :: N_KCH, :],
                start=(i == 0),
                stop=(i == N_KCH - 1),
            )
        e = e_pool.tile([P, C], BF16, tag=etag)
        z = z_pool.tile([P, 1], F32, tag="z" + etag)
        nc.scalar.activation(e[:], g[:], AF.Exp, scale=SCALE, accum_out=z[:])
        r = z_pool.tile([P, 1], F32, tag="r" + etag)
        nc.vector.reciprocal(r[:], z[:])
        es.append(e)
        zs.append(z)
        rs.append(r)
    return es, zs, rs


def build_kernel():
    _patch_tile_drain()
    nc = bass.Bass()
    x_ext = nc.declare_dram_parameter("x", [B_PER_CORE, C, HW], F32, isOutput=False)
    c_ext = nc.declare_dram_parameter(
        "condition", [B_PER_CORE, C, HW], F32, isOutput=False
    )
    out_ext = nc.declare_dram_parameter("out", [B_PER_CORE, C, HW], F32, isOutput=True)

    eye_dram = nc.inline_tensor(np.eye(P, dtype=ml_dtypes.bfloat16), name="eye128")

    with tile.TileContext(nc) as tc:
        with (
            tc.tile_pool(name="const", bufs=1) as const_pool,
            tc.tile_pool(name="xn", bufs=3) as xn_pool,
            tc.tile_pool(name="cn", bufs=3) as cn_pool,
            tc.tile_pool(name="cs", bufs=3) as cs_pool,
            tc.tile_pool(name="xT", bufs=3) as xT_pool,
            tc.tile_pool(name="csT", bufs=3) as csT_pool,
            tc.tile_pool(name="E", bufs=22) as e_pool,
            tc.tile_pool(name="z", bufs=24) as z_pool,
            tc.tile_pool(name="D", bufs=6) as d_pool,
            tc.tile_pool(name="outs", bufs=4) as out_pool,
            tc.tile_pool(name="psum_g", bufs=2, space="PSUM") as psum_g,
            tc.tile_pool(name="psum_u", bufs=3, space="PSUM") as psum_u,
        ):
            eye = const_pool.tile([P, P], BF16)
            nc.sync.dma_start(eye[:], eye_dram[:])

            staged = {}
            grams = {}

            def stage(b):
                """loads + sigmoid-via-tanh + SBUF->SBUF xbar transposes."""
                xTb = xT_pool.tile([P, N_CB * N_KCH, P], BF16, tag="xT")
                csTb = csT_pool.tile([P, N_CB * N_KCH, P], BF16, tag="csT")
                # one cast-DMA for all of x[b]: [128, 4, 896] bf16 (pad unused)
                xnb = xn_pool.tile([P, N_CB, HWP], BF16, tag="xn")
                nc.gpsimd.dma_start(
                    xnb[:, :, :HW],
                    x_ext[b].rearrange("(k p) n -> p k n", p=P),
                )
                nc.sync.dma_start_transpose(
                    xTb[:], xnb.rearrange("p k n -> p (k n)")
                )
                cnb = cn_pool.tile([P, N_CB, HW], BF16, tag="cn")
                nc.gpsimd.dma_start(
                    cnb[:], c_ext[b].rearrange("(k p) n -> p k n", p=P)
                )
                csb = cs_pool.tile([P, N_CB, HWP], BF16, tag="cs")
                # sigmoid(z) = 0.5 * tanh(z/2) + 0.5 (same ACT set as exp)
                ct = cs_pool.tile([P, N_CB, HW], BF16, tag="ct")
                nc.scalar.activation(ct[:], cnb[:], AF.Tanh, scale=0.5)
                nc.vector.tensor_scalar(csb[:, :, :HW], ct[:], 0.5, 0.5, MUL, ADD)
                nc.sync.dma_start_transpose(
                    csTb[:], csb.rearrange("p k n -> p (k n)")
                )
                staged[b] = (xnb, xTb, csTb)

            def gram_stage(b):
                xnb, xTb, csTb = staged.pop(b)
                E1, z1, r1 = _gram_exp(nc, psum_g, xTb, e_pool, z_pool, "e1")
                E2, _z2, r2 = _gram_exp(nc, psum_g, csTb, e_pool, z_pool, "e2")

                # D[cb] = diag(Z1[cb]) as a [128, 128] bf16 tile
                Ds = []
                for cb in range(N_CB):
                    d = d_pool.tile([P, P], BF16, tag="D")
                    nc.scalar.activation(d[:], eye[:], AF.Copy, scale=z1[cb][:])
                    Ds.append(d)
                grams[b] = (xnb, E1, r1, E2, r2, Ds)

            def apply_stage(b):
                xnb, E1, r1, E2, r2, Ds = grams.pop(b)
                obig = out_pool.tile([P, N_CB, HW], BF16, tag="o")
                for cb in range(N_CB):
                    o = obig[:, cb, :]
                    u1 = psum_u.tile([P, 1024], F32, tag="u")
                    u2 = psum_u.tile([P, 1024], F32, tag="u")
                    for n0, nw in APPLY_NSPLIT:
                        for k in range(N_CB):
                            nc.tensor.matmul(
                                u1[:, n0 : n0 + nw],
                                E1[k][:, cb * P : (cb + 1) * P],
                                xnb[:, k, n0 : n0 + nw],
                                start=(k == 0),
                                stop=False,
                            )
                        # residual: diag(Z1) @ X so that u1*r1 includes +X
                        nc.tensor.matmul(
                            u1[:, n0 : n0 + nw],
                            Ds[cb][:],
                            xnb[:, cb, n0 : n0 + nw],
                            start=False,
                            stop=True,
                        )
                        for k in range(N_CB):
                            nc.tensor.matmul(
                                u2[:, n0 : n0 + nw],
                                E2[k][:, cb * P : (cb + 1) * P],
                                xnb[:, k, n0 : n0 + nw],
                                start=(k == 0),
                                stop=(k == N_CB - 1),
                            )
                    t1 = out_pool.tile([P, HW], BF16, tag="t1")
                    nc.vector.tensor_scalar(t1[:], u1[:, :HW], r1[cb][:], None, MUL)
                    t2 = out_pool.tile([P, HW], BF16, tag="t2")
                    nc.scalar.activation(t2[:], u2[:, :HW], AF.Copy, scale=r2[cb][:])
                    nc.vector.tensor_add(o[:], t1[:], t2[:])
                nc.gpsimd.dma_start(
                    out_ext[b].rearrange("(k p) n -> p k n", p=P), obig[:]
                )

            stage(0)
            stage(1)
            gram_stage(0)
            for b in range(B_PER_CORE):
                if b + 2 < B_PER_CORE:
                    stage(b + 2)
                if b + 1 < B_PER_CORE:
                    gram_stage(b + 1)
                apply_stage(b)
    n = _split_multi_waits(nc)
    print(f"[kernel] split {n} multi-wait instructions")
    return nc


_NC_CACHE = None


def kernel(x: np.ndarray, condition: np.ndarray, _trace: bool = False):
    """Full inputs [64, 512, 28, 28] f32 -> full output [64, 512, 784] f32."""
    global _NC_CACHE
    B = x.shape[0]
    xf = np.ascontiguousarray(x.reshape(B, C, HW), dtype=np.float32)
    cf = np.ascontiguousarray(condition.reshape(B, C, HW), dtype=np.float32)

    if _NC_CACHE is None:
        _NC_CACHE = build_kernel()
    nc = _NC_CACHE

    in_maps = [
        {
            "x": xf[i * B_PER_CORE : (i + 1) * B_PER_CORE],
            "condition": cf[i * B_PER_CORE : (i + 1) * B_PER_CORE],
        }
        for i in range(N_CORES)
    ]
    res = run_bass_kernel_spmd(nc, in_maps, core_ids=list(range(N_CORES)), trace=_trace)
    out = np.concatenate([res.results[i]["out"] for i in range(N_CORES)], axis=0)
    kernel.last_result = res
    return out


# revision 43
# speedup vs baseline: 1.1414x; 1.0007x over previous
"""Trainium2 Bass kernel for nn_Attention_75342316306884.

Per-batch channel-channel attention:
  xf = x.reshape(B, C, HW); cf = condition.reshape(B, C, HW)
  w1 = softmax(xf @ xf^T * HW^-0.5); w2 = softmax(sig(cf) @ sig(cf)^T * HW^-0.5)
  out = xf + (w1 + w2) @ xf          -> [B, C, HW] float32

Sharding: pure data parallel, batch dim 64 -> 8 cores x 8 batches.

Per-core pipeline, software-skewed two batches ahead (emission order
stage(b+2), gram(b+1), apply(b) so staging and grams overlap applies in
every engine stream):
  stage: one cast-DMA per tensor (f32 HBM -> bf16 [128, 4, 896] SBUF tile
    via a 3D access pattern; pad cols unused); condition -> sigmoid via
    tanh (sigmoid(z) = 0.5*tanh(z/2) + 0.5 -- tanh lives in the same ACT
    table set as exp: zero ACT table switches in the whole kernel); then
    ONE SBUF->SBUF xbar DMA-transpose per tensor into a [128, 28, 512->128]
    bf16 tile: slice [:, 7*cb + e, :] holds rows n = 128e+p of block cb.
  gram: two 512x512 grams on TensorE (bf16, f32 PSUM accumulate), the
    contraction walking 7 chunk-slices (the moving operand reads the
    transposed tile with a stride-7 middle dim); ACT exp with fused
    per-row accumulation Z; diag(Z1) tiles built on ACT.
  apply: 9+4 matmuls per (c-block) into 2-bank [128, 1024] PSUM tiles with
    the residual folded in as an extra diag(Z1) matmul chunk; epilogue is
    t1 = u1*r1 (DVE), t2 = u2*r2 (ACT), o = t1+t2 (DVE, bf16) and one
    cast-DMA per batch back to f32 HBM.

Key algebraic tricks:
 * G = xf@xf^T is symmetric, so unnormalized E = exp(G*s) is symmetric too;
   softmax(G) @ xf == diag(1/rowsum(E)) @ (E @ xf).  E's stored
   [c-part, d-free] tiles serve directly as the [K=d, M=c] stationary
   operands of the apply matmul -- no attention-matrix transpose, and the
   normalization is a per-partition scalar.  (exp without max-subtraction
   is safe: logits bounded by ~|x|^2/28 ~ 35.)
 * residual: out = (E1 @ X + diag(Z1) @ X) * r1 + (E2 @ X) * r2, so the
   "+ xf" is one extra 128-contraction matmul instead of a VectorE pass.
"""

import sys

import numpy as np

for _p in ("/opt/trn_rl_repo",):
    if _p not in sys.path:
        sys.path.append(_p)

import ml_dtypes

import concourse.bass as bass
import concourse.mybir as mybir
import concourse.tile as tile
from concourse.bass_utils import run_bass_kernel_spmd
from concourse.vector_clock import ScopedClock

F32 = mybir.dt.float32
BF16 = mybir.dt.bfloat16
AF = mybir.ActivationFunctionType
MUL = mybir.AluOpType.mult
ADD = mybir.AluOpType.add

N_CORES = 8
B_PER_CORE = 8
C = 512  # channels
HW = 784  # 28*28
HWP = 896  # padded to 7*128 for the xbar transpose
SCALE = float(HW) ** -0.5
P = 128
N_KCH = 7  # gram contraction chunks: 6x128 + 1x16
KCH_SIZES = (128, 128, 128, 128, 128, 128, 16)
N_CB = 4  # 512 / 128 c-blocks
APPLY_NSPLIT = ((0, 512), (512, 272))


def _patch_tile_drain():
    """walrus codegen in this toolchain rejects >1 sem-wait on one SP CTRL
    (drain/nop) instruction; spread the Tile end-of-context drain waits
    across several nops instead."""
    if getattr(tile.TileContext, "_drain_patched", False):
        return

    def _drain_and_barrier(self, tick_clock, wait_clock):
        absorber = self.nc.sync.nop()
        wait_clock.add_sem_waits(
            absorber.ins, ScopedClock({None: tick_clock.global_clock})
        )
        si = absorber.ins.sync_info
        waits = list(si.on_wait) if si is not None and si.on_wait else []
        if len(waits) > 1:
            absorber.ins.sync_info = mybir.SyncInfo(on_wait=waits[:1], on_update=[])
            for w in waits[1:]:
                n2 = self.nc.sync.nop()
                n2.ins.sync_info = mybir.SyncInfo(on_wait=[w], on_update=[])
        self.nc.sync.drain()
        self.nc.all_engine_barrier()
        assert self.sems is not None
        popped = self.nc._tile_sem_poison_stack.pop()
        assert popped is self._sem_poison
        self.nc.clear_and_free_semaphores(list(self.sems.allocated().values()))
        self.nc.all_engine_barrier()

    tile.TileContext._drain_and_barrier = _drain_and_barrier
    tile.TileContext._drain_patched = True


def _split_multi_waits(nc, limit=1):
    """This walrus build allows only `limit` sem-wait commands per
    instruction.  Hoist excess waits onto same-engine NoOps placed
    immediately before the instruction (per-engine program order makes
    this semantically identical)."""
    n_split = 0
    for f in nc.m.functions:
        for bb in f.blocks:
            new_insts = []
            for inst in bb.instructions:
                si = inst.sync_info
                waits = list(si.on_wait) if si is not None and si.on_wait else []
                if len(waits) > limit:
                    for j, w in enumerate(waits[:-limit]):
                        nop = mybir.InstNoOp(
                            name=f"{inst.name}-wsplit{j}", ins=[], outs=[]
                        )
                        nop.engine = inst.engine
                        nop.sync_info = mybir.SyncInfo(on_wait=[w], on_update=[])
                        new_insts.append(nop)
                    inst.sync_info = mybir.SyncInfo(
                        on_wait=waits[-limit:],
                        on_update=list(si.on_update) if si.on_update else [],
                    )
                    n_split += 1
                new_insts.append(inst)
            if len(new_insts) != len(bb.instructions):
                bb.instructions = new_insts
                assert len(bb.instructions) == len(new_insts)
    return n_split


def _gram_exp(nc, psum_g, opT, e_pool, z_pool, etag):
    """opT: one [128, 7, 512] bf16 tile; chunk e's partition p holds row
    n = 128e + p of the transposed operand (chunk 6: first 16 valid).
    Returns (E, z, r): E = exp(scale*gram) (4 x [128, 512] bf16),
    z = rowsum(E), r = 1/z (each 4 x [128, 1] f32)."""
    es, zs, rs = [], [], []
    for cb in range(N_CB):
        g = psum_g.tile([P, C], F32, tag="g")
        for i, k in enumerate(range(N_KCH)):
            kk = KCH_SIZES[k]
            nc.tensor.matmul(
                g[:],
                opT[:kk, N_KCH * cb + k, :],
                opT[:kk, k :: N_KCH, :],
                start=(i == 0),
                stop=(i == N_KCH - 1),
            )
        e = e_pool.tile([P, C], BF16, tag=etag)
        z = z_pool.tile([P, 1], F32, tag="z" + etag)
        nc.scalar.activation(e[:], g[:], AF.Exp, scale=SCALE, accum_out=z[:])
        r = z_pool.tile([P, 1], F32, tag="r" + etag)
        nc.vector.reciprocal(r[:], z[:])
        es.append(e)
        zs.append(z)
        rs.append(r)
    return es, zs, rs


def build_kernel():
    _patch_tile_drain()
    nc = bass.Bass()
    x_ext = nc.declare_dram_parameter("x", [B_PER_CORE, C, HW], F32, isOutput=False)
    c_ext = nc.declare_dram_parameter(
        "condition", [B_PER_CORE, C, HW], F32, isOutput=False
    )
    out_ext = nc.declare_dram_parameter("out", [B_PER_CORE, C, HW], F32, isOutput=True)

    eye_dram = nc.inline_tensor(np.eye(P, dtype=ml_dtypes.bfloat16), name="eye128")

    with tile.TileContext(nc) as tc:
        with (
            tc.tile_pool(name="const", bufs=1) as const_pool,
            tc.tile_pool(name="xn", bufs=3) as xn_pool,
            tc.tile_pool(name="cn", bufs=3) as cn_pool,
            tc.tile_pool(name="cs", bufs=3) as cs_pool,
            tc.tile_pool(name="xT", bufs=3) as xT_pool,
            tc.tile_pool(name="csT", bufs=3) as csT_pool,
            tc.tile_pool(name="E", bufs=20) as e_pool,
            tc.tile_pool(name="z", bufs=24) as z_pool,
            tc.tile_pool(name="D", bufs=6) as d_pool,
            tc.tile_pool(name="outs", bufs=4) as out_pool,
            tc.tile_pool(name="psum_g", bufs=2, space="PSUM") as psum_g,
            tc.tile_pool(name="psum_u", bufs=3, space="PSUM") as psum_u,
        ):
            eye = const_pool.tile([P, P], BF16)
            nc.sync.dma_start(eye[:], eye_dram[:])

            staged = {}
            grams = {}

            def stage(b):
                """loads + sigmoid-via-tanh + SBUF->SBUF xbar transposes."""
                xTb = xT_pool.tile([P, N_CB * N_KCH, P], BF16, tag="xT")
                csTb = csT_pool.tile([P, N_CB * N_KCH, P], BF16, tag="csT")
                # one cast-DMA for all of x[b]: [128, 4, 896] bf16 (pad unused)
                xnb = xn_pool.tile([P, N_CB, HWP], BF16, tag="xn")
                nc.gpsimd.dma_start(
                    xnb[:, :, :HW],
                    x_ext[b].rearrange("(k p) n -> p k n", p=P),
                )
                nc.sync.dma_start_transpose(
                    xTb[:], xnb.rearrange("p k n -> p (k n)")
                )
                cnb = cn_pool.tile([P, N_CB, HW], BF16, tag="cn")
                nc.gpsimd.dma_start(
                    cnb[:], c_ext[b].rearrange("(k p) n -> p k n", p=P)
                )
                csb = cs_pool.tile([P, N_CB, HWP], BF16, tag="cs")
                # sigmoid(z) = 0.5 * tanh(z/2) + 0.5 (same ACT set as exp)
                ct = cs_pool.tile([P, N_CB, HW], BF16, tag="ct")
                nc.scalar.activation(ct[:], cnb[:], AF.Tanh, scale=0.5)
                nc.vector.tensor_scalar(csb[:, :, :HW], ct[:], 0.5, 0.5, MUL, ADD)
                nc.sync.dma_start_transpose(
                    csTb[:], csb.rearrange("p k n -> p (k n)")
                )
                staged[b] = (xnb, xTb, csTb)

            def gram_stage(b):
                xnb, xTb, csTb = staged.pop(b)
                E1, z1, r1 = _gram_exp(nc, psum_g, xTb, e_pool, z_pool, "e1")
                E2, _z2, r2 = _gram_exp(nc, psum_g, csTb, e_pool, z_pool, "e2")

                # D[cb] = diag(Z1[cb]) as a [128, 128] bf16 tile
                Ds = []
                for cb in range(N_CB):
                    d = d_pool.tile([P, P], BF16, tag="D")
                    nc.scalar.activation(d[:], eye[:], AF.Copy, scale=z1[cb][:])
                    Ds.append(d)
                grams[b] = (xnb, E1, r1, E2, r2, Ds)

            def apply_stage(b):
                xnb, E1, r1, E2, r2, Ds = grams.pop(b)
                obig = out_pool.tile([P, N_CB, HW], BF16, tag="o")
                for cb in range(N_CB):
                    o = obig[:, cb, :]
                    u1 = psum_u.tile([P, 1024], F32, tag="u")
                    u2 = psum_u.tile([P, 1024], F32, tag="u")
                    for n0, nw in APPLY_NSPLIT:
                        for k in range(N_CB):
                            nc.tensor.matmul(
                                u1[:, n0 : n0 + nw],
                                E1[k][:, cb * P : (cb + 1) * P],
                                xnb[:, k, n0 : n0 + nw],
                                start=(k == 0),
                                stop=False,
                            )
                        # residual: diag(Z1) @ X so that u1*r1 includes +X
                        nc.tensor.matmul(
                            u1[:, n0 : n0 + nw],
                            Ds[cb][:],
                            xnb[:, cb, n0 : n0 + nw],
                            start=False,
                            stop=True,
                        )
                        for k in range(N_CB):
                            nc.tensor.matmul(
                                u2[:, n0 : n0 + nw],
                                E2[k][:, cb * P : (cb + 1) * P],
                                xnb[:, k, n0 : n0 + nw],
                                start=(k == 0),
                                stop=(k == N_CB - 1),
                            )
                    t1 = out_pool.tile([P, HW], BF16, tag="t1")
                    nc.vector.tensor_scalar(t1[:], u1[:, :HW], r1[cb][:], None, MUL)
                    t2 = out_pool.tile([P, HW], BF16, tag="t2")
                    nc.scalar.activation(t2[:], u2[:, :HW], AF.Copy, scale=r2[cb][:])
                    nc.vector.tensor_add(o[:], t1[:], t2[:])
                nc.gpsimd.dma_start(
                    out_ext[b].rearrange("(k p) n -> p k n", p=P), obig[:]
                )

            stage(0)
            stage(1)
            gram_stage(0)
            for b in range(B_PER_CORE):
                if b + 2 < B_PER_CORE:
                    stage(b + 2)
                if b + 1 < B_PER_CORE:
                    gram_stage(b + 1)
                apply_stage(b)
    n = _split_multi_waits(nc)
    print(f"[kernel] split {n} multi-wait instructions")
    return nc


_NC_CACHE = None


def kernel(x: np.ndarray, condition: np.ndarray, _trace: bool = False):
    """Full inputs [64, 512, 28, 28] f32 -> full output [64, 512, 784] f32."""
    global _NC_CACHE
    B = x.shape[0]
    xf = np.ascontiguousarray(x.reshape(B, C, HW), dtype=np.float32)
    cf = np.ascontiguousarray(condition.reshape(B, C, HW), dtype=np.float32)

    if _NC_CACHE is None:
        _NC_CACHE = build_kernel()
    nc = _NC_CACHE

    in_maps = [
        {
            "x": xf[i * B_PER_CORE : (i + 1) * B_PER_CORE],
            "condition": cf[i * B_PER_CORE : (i + 1) * B_PER_CORE],
        }
        for i in range(N_CORES)
    ]
    res = run_bass_kernel_spmd(nc, in_maps, core_ids=list(range(N_CORES)), trace=_trace)
    out = np.concatenate([res.results[i]["out"] for i in range(N_CORES)], axis=0)
    kernel.last_result = res
    return out
